# revision 15
# baseline (speedup 1.0000x reference)
"""Trainium2 Bass kernel for nn_ApplyKernel (gnn_message_passing).

Reference computation (Z=4, N=256, CIN=32, COUT=32, HID=64):
    diff[z,a,b,:] = geometry[z,b] - geometry[z,a]
    h = relu(diff @ W1 + b1)                      # [z,a,b,64]
    k = (h @ W2 + b2).reshape(z,n,n,32,32)        # [z,a,b,i,j]
    out = einsum('zabij,zbj->zabi', k, features)  # [z,a,b,32]

Algebraic restructure used here (exact, no approximation):
    g[b,k]   = (geometry[z] @ W1)[b,k]            (tiny matmul)
    h[a,b,k] = relu(g[b,k] + b1[k] - g[a,k])      (pairwise broadcast-sub)
    V[b,k,i] = sum_j W2[k, i*CIN+j] * features[z,b,j]   (contract features
               with W2 FIRST -> 32x fewer FLOPs than materializing k)
    out[a,b,i] = sum_k h[a,b,k] * V[b,k,i] (+ c[b,i] from b2)

Sharding: 8 cores = (z, a-half) blocks. Core c owns z=c//2 and
a in [128*(c%2), 128*(c%2)+128) -> 'a' maps onto the 128 SBUF partitions.
Pure data-parallel; each core writes its contiguous 4 MiB output block.

On-core layout (per core, all fp32):
  - packed[p=(half,k), bp] = g[2bp+half, k] + b1[k]        (128x128 SBUF)
  - ga_neg[p=(half,k), a]  = -g[a0+a, k]                   (128x128 SBUF)
  - h[p=(half,k), bp, a]   = relu(ga_neg + packed[:,bp])   via per-bp
    tensor_scalar (DVE, fp32 2x mode) / activation-Relu-bias (ACT)
  - VBD[p=(half,k), bp, n=(half',i)] = V[2bp+half',k,i]*delta(half,half')
    built by PE matmuls with block-diagonal host-prepped W2/features
    (the block structure zeroes the off-diagonal entries for free)
  - final: per b-pair matmul  lhsT=h[:,bp,:] [128K,128M],
    rhs=VBD[:,bp,:] [128K,64N] -> PSUM [a=128, (half',i)=64], 8 pairs
    per PSUM bank, DMA'd straight from PSUM to DRAM (free dim order
    64*bp+32*half'+i == 32*b+i, i.e. the natural [a, b, i] layout).
"""

import numpy as np

Z, N, CIN, COUT, HID = 4, 256, 32, 32, 64
N_CORES = 8
AH = N // 2  # 128 a-values per core

_CACHE: dict = {}


def _build_nc():
    import concourse.bass as bass  # noqa: F401
    import concourse.tile as tile
    from concourse import bacc, mybir

    f32 = mybir.dt.float32
    bf16 = mybir.dt.bfloat16
    Ident = mybir.ActivationFunctionType.Identity
    Relu = mybir.ActivationFunctionType.Relu
    Copy = mybir.ActivationFunctionType.Copy
    add = mybir.AluOpType.add
    amax = mybir.AluOpType.max

    nc = bacc.Bacc("TRN2", target_bir_lowering=False, debug=False,
                   num_devices=N_CORES)

    gall_d = nc.declare_dram_parameter("gall", [3, N + AH + 2 * HID], bf16,
                                       isOutput=False)
    b1r_d = nc.declare_dram_parameter("b1r", [2 * HID, 1], f32, isOutput=False)
    fbd_d = nc.declare_dram_parameter("fbd", [2 * CIN, N], bf16, isOutput=False)
    W2bd_d = nc.declare_dram_parameter("W2bd", [2 * CIN, COUT * 2 * HID], bf16,
                                       isOutput=False)
    out_d = nc.declare_dram_parameter("out", [AH, N * COUT], f32, isOutput=True)

    NBP = N // 2  # 128 b-pairs

    with tile.TileContext(nc) as tc:
        with (
            tc.tile_pool(name="consts", bufs=1) as consts,
            tc.tile_pool(name="hbuf", bufs=1) as hpool,
            tc.tile_pool(name="vbuf", bufs=1) as vpool_sb,
        ):
            # ---- input DMAs (small g-path tensors first) ----
            gall_s = consts.tile([3, N + AH + 2 * HID], bf16)
            nc.sync.dma_start(gall_s[:], gall_d[:])
            gT_s = gall_s[:, 0:N]
            gaT_s = gall_s[:, N:N + AH]
            W1d_s = gall_s[:, N + AH:]
            b1r_s = consts.tile([2 * HID, 1], f32)
            nc.sync.dma_start(b1r_s[:], b1r_d[:])
            # hoist the ACT function-table load to t~0
            nc.scalar.activation(b1r_s[0:1, :], b1r_s[0:1, :], Relu, scale=1.0)
            fbd_s = consts.tile([2 * CIN, N], bf16)
            nc.gpsimd.dma_start(fbd_s[:], fbd_d[:])
            W2bd_s = consts.tile([2 * CIN, COUT * 2 * HID], bf16)
            nc.gpsimd.dma_start(W2bd_s[:], W2bd_d[:])

            packed = consts.tile([2 * HID, NBP], f32)   # g[2bp+half,k]+b1[k]
            ga_neg = consts.tile([2 * HID, AH], bf16)    # -g[a0+a, k]
            VBD = vpool_sb.tile([2 * HID, NBP * 2 * COUT], bf16)
            h_t = hpool.tile([2 * HID, NBP * AH], bf16)
            h_ap = h_t[:].rearrange("p (bp a) -> p bp a", a=AH)
            vbd_ap = VBD[:].rearrange("p (bp n) -> p bp n", n=2 * COUT)

            # ---- phase 1: g = geom @ W1 (both stacked halves) ----
            # gg2+gga share one PSUM bank; gga stays live so ScalarE can
            # compute its h share straight from PSUM with fused scale=-1.
            gpsum_cm = tc.tile_pool(name="gpsum", bufs=1, space="PSUM")
            gpsum = gpsum_cm.__enter__()
            gtile = gpsum.tile([2 * HID, 512], f32)
            gg2 = gtile[:, 0:N]
            gga = gtile[:, N:N + AH]
            nc.tensor.matmul(gg2, W1d_s, gT_s, start=True, stop=True)
            nc.tensor.matmul(gga, W1d_s, gaT_s, start=True, stop=True)

            # packed[p,bp]: p<64 takes even b, p>=64 takes odd b
            gg2v = gg2.rearrange("p (bp two) -> p two bp", two=2)
            nc.scalar.activation(packed[0:HID, :], gg2v[0:HID, 0, :],
                                 Ident, bias=b1r_s[0:HID, :], scale=1.0)
            nc.scalar.activation(packed[HID:2 * HID, :],
                                 gg2v[HID:2 * HID, 1, :],
                                 Ident, bias=b1r_s[HID:2 * HID, :], scale=1.0)
            nc.scalar.activation(ga_neg[:], gga, Ident, scale=-1.0)

            # ---- phase 2: V via block-diag matmuls, PSUM -> VBD in SBUF ----
            # VBD free layout: [i(32), b(256)]; contiguous copy from each
            # PSUM supertile; the final matmul uses a strided rhs AP.
            w2v = W2bd_s[:].rearrange("p (i m) -> p i m", m=2 * HID)
            with tc.tile_pool(name="vpsum", bufs=2, space="PSUM") as vpsum:
                for grp in range(8):
                    sup = vpsum.tile([2 * HID, 4 * N], f32)
                    for ii in range(4):
                        i = grp * 4 + ii
                        nc.tensor.matmul(sup[:, ii * N:(ii + 1) * N],
                                         w2v[:, i, :], fbd_s[:],
                                         start=True, stop=True)
                    vbd_bi = VBD[:].rearrange("p (b i) -> p i b", i=COUT)
                    dst = vbd_bi[:, grp * 4:(grp + 1) * 4, :]
                    srcv = sup[:].rearrange("p (i b) -> p i b", i=4)
                    if grp % 2 == 0:
                        nc.vector.tensor_copy(dst, srcv)
                    else:
                        nc.scalar.activation(dst, srcv, Copy)
            # rhs view for pair bp: [p, (half', i)] at offset 2*bp*COUT
            vbd_r = VBD[:].rearrange("p (b i) -> p b i", i=COUT)

            # ---- phase 3: h build + per-pair matmuls + copy + DMA out ----
            ostage = vpool_sb.tile([AH, N * COUT], f32)
            with tc.tile_pool(name="opsum", bufs=3, space="PSUM") as opsum:
                for g16 in range(16):
                    bank = opsum.tile([AH, 8 * 2 * COUT], f32)
                    for j in range(8):
                        bp = g16 * 8 + j
                        hs = h_ap[:, bp, :]
                        if bp % 16 < 11:
                            nc.vector.tensor_scalar(hs, ga_neg[:],
                                                    packed[:, bp:bp + 1], 0.0,
                                                    add, amax)
                        else:
                            nc.scalar.activation(hs, gga, Relu,
                                                 bias=packed[:, bp:bp + 1],
                                                 scale=-1.0)
                        nc.tensor.matmul(bank[:, j * 64:(j + 1) * 64],
                                         hs, vbd_r[:, 2 * bp:2 * bp + 2, :],
                                         start=True, stop=True)
                    dsts = ostage[:, g16 * 512:(g16 + 1) * 512]
                    if g16 % 2 == 0:
                        nc.vector.tensor_copy(dsts, bank[:])
                    else:
                        nc.scalar.activation(dsts, bank[:], Copy)
                    if g16 % 4 == 3:
                        lo = (g16 - 3) * 512
                        nc.sync.dma_start(out_d[:, lo:lo + 2048],
                                          ostage[:, lo:lo + 2048])
            gpsum_cm.__exit__(None, None, None)
    return nc


def _prep_in_maps(features, geometry, W1, b1, W2):
    import ml_dtypes

    bf = ml_dtypes.bfloat16
    in_maps = []
    for c in range(N_CORES):
        z, half = c // 2, c % 2
        a0 = half * AH
        gT = np.ascontiguousarray(geometry[z].T)                  # [3, 256]
        gaT = np.ascontiguousarray(geometry[z, a0:a0 + AH].T)     # [3, 128]
        W1d = np.ascontiguousarray(np.concatenate([W1, W1], axis=1))
        b1r = np.concatenate([b1, b1])[:, None].copy()
        fT = features[z].T                                        # [32, 256]
        fbd = np.zeros((2 * CIN, N), np.float32)
        fbd[0:CIN, 0::2] = fT[:, 0::2]
        fbd[CIN:2 * CIN, 1::2] = fT[:, 1::2]
        base = W2.reshape(HID, COUT, CIN).transpose(2, 1, 0)      # [j, i, k]
        W2bd = np.zeros((2 * CIN, COUT, 2 * HID), np.float32)
        W2bd[0:CIN, :, 0:HID] = base
        W2bd[CIN:2 * CIN, :, HID:2 * HID] = base
        in_maps.append({
            "gall": np.concatenate([gT, gaT, W1d], axis=1).astype(bf),
            "b1r": b1r.astype(np.float32),
            "fbd": fbd.astype(bf),
            "W2bd": np.ascontiguousarray(W2bd.reshape(2 * CIN, -1)).astype(bf),
        })
    return in_maps


def _patch_ldw_opt():
    """walrus is invoked with --enable-ldw-opt=false hardcoded; flip it so
    LDWEIGHTS can target the background weight buffer and overlap matmuls."""
    import concourse.bass_utils as bu

    if getattr(bu.run_command, "_ldw_patched", False):
        return
    orig = bu.run_command

    def patched(cmd, **kw):
        if isinstance(cmd, list):
            cmd = ["--enable-ldw-opt=true" if c == "--enable-ldw-opt=false"
                   else c for c in cmd]
        return orig(cmd, **kw)

    patched._ldw_patched = True
    bu.run_command = patched


def _run(features, geometry, W1, b1, W2, b2, trace=False):
    from concourse.bass_utils import run_bass_kernel_spmd

    if "nc" not in _CACHE:
        nc = _build_nc()
        if not nc.is_finalized():
            nc.finalize()
        _CACHE["nc"] = nc
    nc = _CACHE["nc"]
    in_maps = _prep_in_maps(features, geometry, W1, b1, W2)
    res = run_bass_kernel_spmd(nc, in_maps, list(range(N_CORES)), trace=trace)
    out = np.empty((Z, N, N, COUT), np.float32)
    for c in range(N_CORES):
        z, half = c // 2, c % 2
        a0 = half * AH
        out[z, a0:a0 + AH] = res.results[c]["out"].reshape(AH, N, COUT)
    if b2 is not None and np.any(b2):
        # b2 is zero in the reference's setup_inputs; general-case fallback.
        cbi = features @ b2.reshape(COUT, CIN).T          # [z, b, i]
        out += cbi[:, None, :, :]
    return out, res


def kernel(features, geometry, W1, b1, W2, b2):
    out, _ = _run(np.asarray(features), np.asarray(geometry), np.asarray(W1),
                  np.asarray(b1), np.asarray(W2), np.asarray(b2))
    return out


# revision 17
# speedup vs baseline: 1.4666x; 1.4666x over previous
"""Trainium2 Bass kernel for nn_ApplyKernel (gnn_message_passing).

Reference computation (Z=4, N=256, CIN=32, COUT=32, HID=64):
    diff[z,a,b,:] = geometry[z,b] - geometry[z,a]
    h = relu(diff @ W1 + b1)                      # [z,a,b,64]
    k = (h @ W2 + b2).reshape(z,n,n,32,32)        # [z,a,b,i,j]
    out = einsum('zabij,zbj->zabi', k, features)  # [z,a,b,32]

Algebraic restructure used here (exact, no approximation):
    g[b,k]   = (geometry[z] @ W1)[b,k]            (tiny matmul)
    h[a,b,k] = relu(g[b,k] + b1[k] - g[a,k])      (pairwise broadcast-sub)
    V[b,k,i] = sum_j W2[k, i*CIN+j] * features[z,b,j]   (contract features
               with W2 FIRST -> 32x fewer FLOPs than materializing k)
    out[a,b,i] = sum_k h[a,b,k] * V[b,k,i] (+ c[b,i] from b2)

Sharding: 8 cores = (z, a-half) blocks. Core c owns z=c//2 and
a in [128*(c%2), 128*(c%2)+128) -> 'a' maps onto the 128 SBUF partitions.
Pure data-parallel; each core writes its contiguous 4 MiB output block.

On-core layout (per core, all fp32):
  - packed[p=(half,k), bp] = g[2bp+half, k] + b1[k]        (128x128 SBUF)
  - ga_neg[p=(half,k), a]  = -g[a0+a, k]                   (128x128 SBUF)
  - h[p=(half,k), bp, a]   = relu(ga_neg + packed[:,bp])   via per-bp
    tensor_scalar (DVE, fp32 2x mode) / activation-Relu-bias (ACT)
  - VBD[p=(half,k), bp, n=(half',i)] = V[2bp+half',k,i]*delta(half,half')
    built by PE matmuls with block-diagonal host-prepped W2/features
    (the block structure zeroes the off-diagonal entries for free)
  - final: per b-pair matmul  lhsT=h[:,bp,:] [128K,128M],
    rhs=VBD[:,bp,:] [128K,64N] -> PSUM [a=128, (half',i)=64], 8 pairs
    per PSUM bank, DMA'd straight from PSUM to DRAM (free dim order
    64*bp+32*half'+i == 32*b+i, i.e. the natural [a, b, i] layout).
"""

import numpy as np

Z, N, CIN, COUT, HID = 4, 256, 32, 32, 64
N_CORES = 8
AH = N // 2  # 128 a-values per core

_CACHE: dict = {}


def _build_nc():
    import concourse.bass as bass  # noqa: F401
    import concourse.tile as tile
    from concourse import bacc, mybir

    f32 = mybir.dt.float32
    bf16 = mybir.dt.bfloat16
    Ident = mybir.ActivationFunctionType.Identity
    Relu = mybir.ActivationFunctionType.Relu
    Copy = mybir.ActivationFunctionType.Copy
    add = mybir.AluOpType.add
    amax = mybir.AluOpType.max

    nc = bacc.Bacc("TRN2", target_bir_lowering=False, debug=False,
                   num_devices=N_CORES)

    gall_d = nc.declare_dram_parameter("gall", [3, N + AH + 2 * HID], bf16,
                                       isOutput=False)
    b1r_d = nc.declare_dram_parameter("b1r", [2 * HID, 1], f32, isOutput=False)
    fbd_d = nc.declare_dram_parameter("fbd", [2 * CIN, N], bf16, isOutput=False)
    W2bd_d = nc.declare_dram_parameter("W2bd", [2 * CIN, COUT * 2 * HID], bf16,
                                       isOutput=False)
    out_d = nc.declare_dram_parameter("out", [AH, N * COUT], f32, isOutput=True)

    NBP = N // 2  # 128 b-pairs

    with tile.TileContext(nc) as tc:
        with (
            tc.tile_pool(name="consts", bufs=1) as consts,
            tc.tile_pool(name="hbuf", bufs=1) as hpool,
            tc.tile_pool(name="vbuf", bufs=1) as vpool_sb,
        ):
            # ---- input DMAs (small g-path tensors first) ----
            gall_s = consts.tile([3, N + AH + 2 * HID], bf16)
            nc.sync.dma_start(gall_s[:], gall_d[:])
            gT_s = gall_s[:, 0:N]
            gaT_s = gall_s[:, N:N + AH]
            W1d_s = gall_s[:, N + AH:]
            b1r_s = consts.tile([2 * HID, 1], f32)
            nc.sync.dma_start(b1r_s[:], b1r_d[:])
            # hoist the ACT function-table load to t~0
            nc.scalar.activation(b1r_s[0:1, :], b1r_s[0:1, :], Relu, scale=1.0)
            fbd_s = consts.tile([2 * CIN, N], bf16)
            nc.gpsimd.dma_start(fbd_s[:], fbd_d[:])
            W2bd_s = consts.tile([2 * CIN, COUT * 2 * HID], bf16)
            nc.gpsimd.dma_start(W2bd_s[:], W2bd_d[:])

            packed = consts.tile([2 * HID, NBP], f32)   # g[2bp+half,k]+b1[k]
            ga_neg = consts.tile([2 * HID, AH], bf16)    # -g[a0+a, k]
            VBD = vpool_sb.tile([2 * HID, NBP * 2 * COUT], bf16)
            h_t = hpool.tile([2 * HID, NBP * AH], bf16)
            h_ap = h_t[:].rearrange("p (bp a) -> p bp a", a=AH)
            vbd_ap = VBD[:].rearrange("p (bp n) -> p bp n", n=2 * COUT)

            # ---- phase 1: g = geom @ W1 (both stacked halves) ----
            # gg2+gga share one PSUM bank; gga stays live so ScalarE can
            # compute its h share straight from PSUM with fused scale=-1.
            gpsum_cm = tc.tile_pool(name="gpsum", bufs=1, space="PSUM")
            gpsum = gpsum_cm.__enter__()
            gtile = gpsum.tile([2 * HID, 512], f32)
            gg2 = gtile[:, 0:N]
            gga = gtile[:, N:N + AH]
            nc.tensor.matmul(gg2, W1d_s, gT_s, start=True, stop=True)
            nc.tensor.matmul(gga, W1d_s, gaT_s, start=True, stop=True)

            # packed[p,bp]: p<64 takes even b, p>=64 takes odd b
            gg2v = gg2.rearrange("p (bp two) -> p two bp", two=2)
            nc.scalar.activation(packed[0:HID, :], gg2v[0:HID, 0, :],
                                 Ident, bias=b1r_s[0:HID, :], scale=1.0)
            nc.scalar.activation(packed[HID:2 * HID, :],
                                 gg2v[HID:2 * HID, 1, :],
                                 Ident, bias=b1r_s[HID:2 * HID, :], scale=1.0)
            nc.scalar.activation(ga_neg[:], gga, Ident, scale=-1.0)

            # ---- phase 2: V via block-diag matmuls, PSUM -> VBD in SBUF ----
            # VBD free layout: [i(32), b(256)]; contiguous copy from each
            # PSUM supertile; the final matmul uses a strided rhs AP.
            w2v = W2bd_s[:].rearrange("p (i m) -> p i m", m=2 * HID)
            vbd_ib = VBD[:].rearrange("p (i b) -> p i b", b=N)
            with tc.tile_pool(name="vpsum", bufs=2, space="PSUM") as vpsum:
                for bh in range(2):
                    for iq in range(4):
                        sup = vpsum.tile([2 * HID, 8 * AH], f32)
                        for ii in range(8):
                            i = iq * 8 + ii
                            nc.tensor.matmul(
                                sup[:, ii * AH:(ii + 1) * AH], w2v[:, i, :],
                                fbd_s[:, bh * AH:(bh + 1) * AH],
                                start=True, stop=True)
                        dst = vbd_ib[:, iq * 8:(iq + 1) * 8,
                                     bh * AH:(bh + 1) * AH]
                        srcv = sup[:].rearrange("p (i b) -> p i b", i=8)
                        if iq % 2 == 0:
                            nc.vector.tensor_copy(dst, srcv)
                        else:
                            nc.scalar.activation(dst, srcv, Copy)
            # rhs view for pair bp: [p, (half', i)] at offset 2*bp
            vbd_r = VBD[:].rearrange("p (i b) -> p b i", i=COUT)

            # ---- phase 3: h build + per-pair matmuls + copy + DMA out ----
            ostage = vpool_sb.tile([AH, N * COUT], f32)
            with tc.tile_pool(name="opsum", bufs=3, space="PSUM") as opsum:
                for g16 in range(16):
                    bank = opsum.tile([AH, 8 * 2 * COUT], f32)
                    for j in range(8):
                        bp = g16 * 8 + j
                        hs = h_ap[:, bp, :]
                        if bp % 16 < 11:
                            nc.vector.tensor_scalar(hs, ga_neg[:],
                                                    packed[:, bp:bp + 1], 0.0,
                                                    add, amax)
                        else:
                            nc.scalar.activation(hs, gga, Relu,
                                                 bias=packed[:, bp:bp + 1],
                                                 scale=-1.0)
                        nc.tensor.matmul(bank[:, j * 64:(j + 1) * 64],
                                         hs, vbd_r[:, 2 * bp:2 * bp + 2, :],
                                         start=True, stop=True)
                    dsts = ostage[:, g16 * 512:(g16 + 1) * 512]
                    if g16 % 2 == 0:
                        nc.vector.tensor_copy(dsts, bank[:])
                    else:
                        nc.scalar.activation(dsts, bank[:], Copy)
                    if g16 % 4 == 3:
                        lo = (g16 - 3) * 512
                        nc.sync.dma_start(out_d[:, lo:lo + 2048],
                                          ostage[:, lo:lo + 2048])
            gpsum_cm.__exit__(None, None, None)
    return nc


def _prep_in_maps(features, geometry, W1, b1, W2):
    import ml_dtypes

    bf = ml_dtypes.bfloat16
    in_maps = []
    for c in range(N_CORES):
        z, half = c // 2, c % 2
        a0 = half * AH
        gT = np.ascontiguousarray(geometry[z].T)                  # [3, 256]
        gaT = np.ascontiguousarray(geometry[z, a0:a0 + AH].T)     # [3, 128]
        W1d = np.ascontiguousarray(np.concatenate([W1, W1], axis=1))
        b1r = np.concatenate([b1, b1])[:, None].copy()
        fT = features[z].T                                        # [32, 256]
        fbd = np.zeros((2 * CIN, N), np.float32)
        fbd[0:CIN, 0::2] = fT[:, 0::2]
        fbd[CIN:2 * CIN, 1::2] = fT[:, 1::2]
        base = W2.reshape(HID, COUT, CIN).transpose(2, 1, 0)      # [j, i, k]
        W2bd = np.zeros((2 * CIN, COUT, 2 * HID), np.float32)
        W2bd[0:CIN, :, 0:HID] = base
        W2bd[CIN:2 * CIN, :, HID:2 * HID] = base
        in_maps.append({
            "gall": np.concatenate([gT, gaT, W1d], axis=1).astype(bf),
            "b1r": b1r.astype(np.float32),
            "fbd": fbd.astype(bf),
            "W2bd": np.ascontiguousarray(W2bd.reshape(2 * CIN, -1)).astype(bf),
        })
    return in_maps


def _patch_ldw_opt():
    """walrus is invoked with --enable-ldw-opt=false hardcoded; flip it so
    LDWEIGHTS can target the background weight buffer and overlap matmuls."""
    import concourse.bass_utils as bu

    if getattr(bu.run_command, "_ldw_patched", False):
        return
    orig = bu.run_command

    def patched(cmd, **kw):
        if isinstance(cmd, list):
            cmd = ["--enable-ldw-opt=true" if c == "--enable-ldw-opt=false"
                   else c for c in cmd]
        return orig(cmd, **kw)

    patched._ldw_patched = True
    bu.run_command = patched


def _run(features, geometry, W1, b1, W2, b2, trace=False):
    from concourse.bass_utils import run_bass_kernel_spmd

    if "nc" not in _CACHE:
        nc = _build_nc()
        if not nc.is_finalized():
            nc.finalize()
        _CACHE["nc"] = nc
    nc = _CACHE["nc"]
    in_maps = _prep_in_maps(features, geometry, W1, b1, W2)
    res = run_bass_kernel_spmd(nc, in_maps, list(range(N_CORES)), trace=trace)
    out = np.empty((Z, N, N, COUT), np.float32)
    for c in range(N_CORES):
        z, half = c // 2, c % 2
        a0 = half * AH
        out[z, a0:a0 + AH] = res.results[c]["out"].reshape(AH, N, COUT)
    if b2 is not None and np.any(b2):
        # b2 is zero in the reference's setup_inputs; general-case fallback.
        cbi = features @ b2.reshape(COUT, CIN).T          # [z, b, i]
        out += cbi[:, None, :, :]
    return out, res


def kernel(features, geometry, W1, b1, W2, b2):
    out, _ = _run(np.asarray(features), np.asarray(geometry), np.asarray(W1),
                  np.asarray(b1), np.asarray(W2), np.asarray(b2))
    return out


# revision 18
# speedup vs baseline: 1.4706x; 1.0027x over previous
"""Trainium2 Bass kernel for nn_ApplyKernel (gnn_message_passing).

Reference computation (Z=4, N=256, CIN=32, COUT=32, HID=64):
    diff[z,a,b,:] = geometry[z,b] - geometry[z,a]
    h = relu(diff @ W1 + b1)                      # [z,a,b,64]
    k = (h @ W2 + b2).reshape(z,n,n,32,32)        # [z,a,b,i,j]
    out = einsum('zabij,zbj->zabi', k, features)  # [z,a,b,32]

Algebraic restructure used here (exact, no approximation):
    g[b,k]   = (geometry[z] @ W1)[b,k]            (tiny matmul)
    h[a,b,k] = relu(g[b,k] + b1[k] - g[a,k])      (pairwise broadcast-sub)
    V[b,k,i] = sum_j W2[k, i*CIN+j] * features[z,b,j]   (contract features
               with W2 FIRST -> 32x fewer FLOPs than materializing k)
    out[a,b,i] = sum_k h[a,b,k] * V[b,k,i] (+ c[b,i] from b2)

Sharding: 8 cores = (z, a-half) blocks. Core c owns z=c//2 and
a in [128*(c%2), 128*(c%2)+128) -> 'a' maps onto the 128 SBUF partitions.
Pure data-parallel; each core writes its contiguous 4 MiB output block.

On-core layout (per core, all fp32):
  - packed[p=(half,k), bp] = g[2bp+half, k] + b1[k]        (128x128 SBUF)
  - ga_neg[p=(half,k), a]  = -g[a0+a, k]                   (128x128 SBUF)
  - h[p=(half,k), bp, a]   = relu(ga_neg + packed[:,bp])   via per-bp
    tensor_scalar (DVE, fp32 2x mode) / activation-Relu-bias (ACT)
  - VBD[p=(half,k), bp, n=(half',i)] = V[2bp+half',k,i]*delta(half,half')
    built by PE matmuls with block-diagonal host-prepped W2/features
    (the block structure zeroes the off-diagonal entries for free)
  - final: per b-pair matmul  lhsT=h[:,bp,:] [128K,128M],
    rhs=VBD[:,bp,:] [128K,64N] -> PSUM [a=128, (half',i)=64], 8 pairs
    per PSUM bank, DMA'd straight from PSUM to DRAM (free dim order
    64*bp+32*half'+i == 32*b+i, i.e. the natural [a, b, i] layout).
"""

import numpy as np

Z, N, CIN, COUT, HID = 4, 256, 32, 32, 64
N_CORES = 8
AH = N // 2  # 128 a-values per core

_CACHE: dict = {}


def _build_nc():
    import concourse.bass as bass  # noqa: F401
    import concourse.tile as tile
    from concourse import bacc, mybir

    f32 = mybir.dt.float32
    bf16 = mybir.dt.bfloat16
    Ident = mybir.ActivationFunctionType.Identity
    Relu = mybir.ActivationFunctionType.Relu
    Copy = mybir.ActivationFunctionType.Copy
    add = mybir.AluOpType.add
    amax = mybir.AluOpType.max

    nc = bacc.Bacc("TRN2", target_bir_lowering=False, debug=False,
                   num_devices=N_CORES)

    gall_d = nc.declare_dram_parameter("gall", [3, N + AH + 2 * HID], bf16,
                                       isOutput=False)
    b1r_d = nc.declare_dram_parameter("b1r", [2 * HID, 1], f32, isOutput=False)
    fbd_d = nc.declare_dram_parameter("fbd", [2 * CIN, N], bf16, isOutput=False)
    W2bd_d = nc.declare_dram_parameter("W2bd", [2 * CIN, COUT * 2 * HID], bf16,
                                       isOutput=False)
    out_d = nc.declare_dram_parameter("out", [AH, N * COUT], f32, isOutput=True)

    NBP = N // 2  # 128 b-pairs

    with tile.TileContext(nc) as tc:
        with (
            tc.tile_pool(name="consts", bufs=1) as consts,
            tc.tile_pool(name="hbuf", bufs=1) as hpool,
            tc.tile_pool(name="vbuf", bufs=1) as vpool_sb,
        ):
            # ---- input DMAs (small g-path tensors first) ----
            gall_s = consts.tile([3, N + AH + 2 * HID], bf16)
            nc.sync.dma_start(gall_s[:], gall_d[:])
            gT_s = gall_s[:, 0:N]
            gaT_s = gall_s[:, N:N + AH]
            W1d_s = gall_s[:, N + AH:]
            b1r_s = consts.tile([2 * HID, 1], f32)
            nc.sync.dma_start(b1r_s[:], b1r_d[:])
            # hoist the ACT function-table load to t~0
            nc.scalar.activation(b1r_s[0:1, :], b1r_s[0:1, :], Relu, scale=1.0)
            fbd_s = consts.tile([2 * CIN, N], bf16)
            nc.sync.dma_start(fbd_s[:], fbd_d[:])
            W2bd_s = consts.tile([2 * CIN, COUT * 2 * HID], bf16)
            nc.scalar.dma_start(W2bd_s[:], W2bd_d[:])

            packed = consts.tile([2 * HID, NBP], f32)   # g[2bp+half,k]+b1[k]
            ga_neg = consts.tile([2 * HID, AH], bf16)    # -g[a0+a, k]
            VBD = vpool_sb.tile([2 * HID, NBP * 2 * COUT], bf16)
            h_t = hpool.tile([2 * HID, NBP * AH], bf16)
            h_ap = h_t[:].rearrange("p (bp a) -> p bp a", a=AH)
            vbd_ap = VBD[:].rearrange("p (bp n) -> p bp n", n=2 * COUT)

            # ---- phase 1: g = geom @ W1 (both stacked halves) ----
            # gg2+gga share one PSUM bank; gga stays live so ScalarE can
            # compute its h share straight from PSUM with fused scale=-1.
            gpsum_cm = tc.tile_pool(name="gpsum", bufs=1, space="PSUM")
            gpsum = gpsum_cm.__enter__()
            gtile = gpsum.tile([2 * HID, 512], f32)
            gg2 = gtile[:, 0:N]
            gga = gtile[:, N:N + AH]
            nc.tensor.matmul(gg2, W1d_s, gT_s, start=True, stop=True)
            nc.tensor.matmul(gga, W1d_s, gaT_s, start=True, stop=True)

            # packed[p,bp]: p<64 takes even b, p>=64 takes odd b
            gg2v = gg2.rearrange("p (bp two) -> p two bp", two=2)
            nc.scalar.activation(packed[0:HID, :], gg2v[0:HID, 0, :],
                                 Ident, bias=b1r_s[0:HID, :], scale=1.0)
            nc.scalar.activation(packed[HID:2 * HID, :],
                                 gg2v[HID:2 * HID, 1, :],
                                 Ident, bias=b1r_s[HID:2 * HID, :], scale=1.0)
            nc.scalar.activation(ga_neg[:], gga, Ident, scale=-1.0)

            # ---- phase 2: V via block-diag matmuls, PSUM -> VBD in SBUF ----
            # VBD free layout: [i(32), b(256)]; contiguous copy from each
            # PSUM supertile; the final matmul uses a strided rhs AP.
            w2v = W2bd_s[:].rearrange("p (i m) -> p i m", m=2 * HID)
            vbd_ib = VBD[:].rearrange("p (i b) -> p i b", b=N)
            with tc.tile_pool(name="vpsum", bufs=2, space="PSUM") as vpsum:
                for bh in range(2):
                    for iq in range(4):
                        sup = vpsum.tile([2 * HID, 8 * AH], f32)
                        for ii in range(8):
                            i = iq * 8 + ii
                            nc.tensor.matmul(
                                sup[:, ii * AH:(ii + 1) * AH], w2v[:, i, :],
                                fbd_s[:, bh * AH:(bh + 1) * AH],
                                start=True, stop=True)
                        dst = vbd_ib[:, iq * 8:(iq + 1) * 8,
                                     bh * AH:(bh + 1) * AH]
                        srcv = sup[:].rearrange("p (i b) -> p i b", i=8)
                        if iq % 2 == 0:
                            nc.vector.tensor_copy(dst, srcv)
                        else:
                            nc.scalar.activation(dst, srcv, Copy)
            # rhs view for pair bp: [p, (half', i)] at offset 2*bp
            vbd_r = VBD[:].rearrange("p (i b) -> p b i", i=COUT)

            # ---- phase 3: h build + per-pair matmuls + copy + DMA out ----
            ostage = vpool_sb.tile([AH, N * COUT], f32)
            with tc.tile_pool(name="opsum", bufs=3, space="PSUM") as opsum:
                for g16 in range(16):
                    bank = opsum.tile([AH, 8 * 2 * COUT], f32)
                    for j in range(8):
                        bp = g16 * 8 + j
                        hs = h_ap[:, bp, :]
                        if bp % 16 < 11:
                            nc.vector.tensor_scalar(hs, ga_neg[:],
                                                    packed[:, bp:bp + 1], 0.0,
                                                    add, amax)
                        else:
                            nc.scalar.activation(hs, gga, Relu,
                                                 bias=packed[:, bp:bp + 1],
                                                 scale=-1.0)
                        nc.tensor.matmul(bank[:, j * 64:(j + 1) * 64],
                                         hs, vbd_r[:, 2 * bp:2 * bp + 2, :],
                                         start=True, stop=True)
                    dsts = ostage[:, g16 * 512:(g16 + 1) * 512]
                    if g16 % 2 == 0:
                        nc.vector.tensor_copy(dsts, bank[:])
                    else:
                        nc.scalar.activation(dsts, bank[:], Copy)
                    if g16 % 2 == 1:
                        lo = (g16 - 1) * 512
                        nc.sync.dma_start(out_d[:, lo:lo + 1024],
                                          ostage[:, lo:lo + 1024])
            gpsum_cm.__exit__(None, None, None)
    return nc


def _prep_in_maps(features, geometry, W1, b1, W2):
    import ml_dtypes

    bf = ml_dtypes.bfloat16
    in_maps = []
    for c in range(N_CORES):
        z, half = c // 2, c % 2
        a0 = half * AH
        gT = np.ascontiguousarray(geometry[z].T)                  # [3, 256]
        gaT = np.ascontiguousarray(geometry[z, a0:a0 + AH].T)     # [3, 128]
        W1d = np.ascontiguousarray(np.concatenate([W1, W1], axis=1))
        b1r = np.concatenate([b1, b1])[:, None].copy()
        fT = features[z].T                                        # [32, 256]
        fbd = np.zeros((2 * CIN, N), np.float32)
        fbd[0:CIN, 0::2] = fT[:, 0::2]
        fbd[CIN:2 * CIN, 1::2] = fT[:, 1::2]
        base = W2.reshape(HID, COUT, CIN).transpose(2, 1, 0)      # [j, i, k]
        W2bd = np.zeros((2 * CIN, COUT, 2 * HID), np.float32)
        W2bd[0:CIN, :, 0:HID] = base
        W2bd[CIN:2 * CIN, :, HID:2 * HID] = base
        in_maps.append({
            "gall": np.concatenate([gT, gaT, W1d], axis=1).astype(bf),
            "b1r": b1r.astype(np.float32),
            "fbd": fbd.astype(bf),
            "W2bd": np.ascontiguousarray(W2bd.reshape(2 * CIN, -1)).astype(bf),
        })
    return in_maps


def _patch_ldw_opt():
    """walrus is invoked with --enable-ldw-opt=false hardcoded; flip it so
    LDWEIGHTS can target the background weight buffer and overlap matmuls."""
    import concourse.bass_utils as bu

    if getattr(bu.run_command, "_ldw_patched", False):
        return
    orig = bu.run_command

    def patched(cmd, **kw):
        if isinstance(cmd, list):
            cmd = ["--enable-ldw-opt=true" if c == "--enable-ldw-opt=false"
                   else c for c in cmd]
        return orig(cmd, **kw)

    patched._ldw_patched = True
    bu.run_command = patched


def _run(features, geometry, W1, b1, W2, b2, trace=False):
    from concourse.bass_utils import run_bass_kernel_spmd

    if "nc" not in _CACHE:
        nc = _build_nc()
        if not nc.is_finalized():
            nc.finalize()
        _CACHE["nc"] = nc
    nc = _CACHE["nc"]
    in_maps = _prep_in_maps(features, geometry, W1, b1, W2)
    res = run_bass_kernel_spmd(nc, in_maps, list(range(N_CORES)), trace=trace)
    out = np.empty((Z, N, N, COUT), np.float32)
    for c in range(N_CORES):
        z, half = c // 2, c % 2
        a0 = half * AH
        out[z, a0:a0 + AH] = res.results[c]["out"].reshape(AH, N, COUT)
    if b2 is not None and np.any(b2):
        # b2 is zero in the reference's setup_inputs; general-case fallback.
        cbi = features @ b2.reshape(COUT, CIN).T          # [z, b, i]
        out += cbi[:, None, :, :]
    return out, res


def kernel(features, geometry, W1, b1, W2, b2):
    out, _ = _run(np.asarray(features), np.asarray(geometry), np.asarray(W1),
                  np.asarray(b1), np.asarray(W2), np.asarray(b2))
    return out
